# revision 34
# baseline (speedup 1.0000x reference)
"""Trainium2 Bass kernel for nn_Attention_23055384445157.

Causal multi-head attention block (fp32 reference):
  qkv = x @ w_qkv; split heads; q *= 1/sqrt(64)
  sim = q k^T  (causal masked; key mask is all-ones by construction)
  attn = softmax(sim); out = attn @ v; out = out @ w_out; layernorm(out) * g

Shapes: x [2, 2048, 1024], 16 heads x 64 dims, w_qkv [1024, 3072],
w_out [1024, 1024], g [1024]. Output [2, 2048, 1024] fp32.

Sharding across 8 NeuronCores (SPMD, one program):
  Core c computes heads {2c, 2c+1} for BOTH batches:
    - Q^T/K^T [128 = 2 heads x 64 dims, 2048 seq] and V [2048, 2x65] per
      batch via f16 matmuls from x^T (V column 64 is a ones column whose
      AV product accumulates the softmax denominator for free)
    - per (batch, q-chunk of 512, key tile): scores S^T[k, q] for both
      heads in one [128, 1024] PSUM tile, exp on the causally-valid
      columns only, tm mask on the diagonal band
    - AV in [q, d] layout: per (head, q-128-chunk) matmul with the attn
      tile as stationary and V' as moving -> [128 q, 65] PSUM at full
      partition utilization (the [d, q] layout only fills 65/128 rows)
    - softmax normalization BEFORE the collective: reciprocal of the
      denominator column, per-partition scalar multiply, then PE
      transposes back to [d, q] for the out-projection contraction
  One global 8-way AllToAll redistributes normalized attn^T (128-row
  blocks) from (head-sharded, all queries) to (query-sharded, all
  heads): core c ends with data for batch c//4, query rows 512*(c%4)..+512.
  Stage D is then just out-proj [512, 1024] @ w_out and layernorm; the
  host concatenates core outputs.

All matmul operands are float16 (~5e-4 relative error per element, well
inside the 2e-2 gate; PE runs f16 at full rate and FWL halves weight-load
time). PSUM accumulation is fp32 throughout. Emission interleaves each
projection chunk with the previous attention q-chunk (engines execute
in order, and B(b, qc) only needs x(b) columns 0..512*(qc+1)) so the
in-order PE queue stays fed during the exp-bound attention phase.
"""

import numpy as np

import concourse.mybir as mybir
import concourse.tile as tile
from concourse import bacc
from concourse import bass_utils

P = 128
B = 2
SEQ = 2048
DIM = 1024
DH = 64
HEADS = 16
H_PER_CORE = 2
N_CORES = 8
KD = DIM // P          # 8 contraction chunks
NKT = SEQ // P         # 16 key tiles
NQC = SEQ // 512       # 4 query chunks of 512
INNER_C = H_PER_CORE * DH  # 128 inner dims per core
SCALE = DH ** -0.5
EPS = 1e-5

f32 = mybir.dt.float32
f16 = mybir.dt.float16
AX = mybir.AxisListType.X
EXP = mybir.ActivationFunctionType.Exp
IDENT = mybir.ActivationFunctionType.Identity


def build_nc(use_collective=True, num_devices=N_CORES, reps=1, apply_g=False):
    nc = bacc.Bacc(
        "TRN2", target_bir_lowering=False, debug=False, num_devices=num_devices
    )

    xT = [
        nc.dram_tensor(f"xT{b}", [DIM, SEQ], f16, kind="ExternalInput").ap()
        for b in range(B)
    ]
    wq_d = nc.dram_tensor("wq", [P, KD, INNER_C], f16, kind="ExternalInput").ap()
    wk_d = nc.dram_tensor("wk", [P, KD, INNER_C], f16, kind="ExternalInput").ap()
    wv_d = nc.dram_tensor("wv", [P, KD, INNER_C], f16, kind="ExternalInput").ap()
    wo_d = nc.dram_tensor("wo", [P, KD, DIM], f16, kind="ExternalInput").ap()
    tm_d = nc.dram_tensor("tm", [P, P], f16, kind="ExternalInput").ap()
    idm_d = nc.dram_tensor("idm", [P, P], f16, kind="ExternalInput").ap()
    if apply_g:
        g_d = nc.dram_tensor("g", [DIM], f32, kind="ExternalInput").ap()
    out_d = nc.dram_tensor("out", [512, DIM], f32, kind="ExternalOutput").ap()

    import contextlib

    with tile.TileContext(nc) as tc:
      for _rep in range(reps):
        with (
            tc.tile_pool(name="const", bufs=1) as cpool,
            tc.tile_pool(name="proj", bufs=1) as proj,
            tc.tile_pool(name="pt", bufs=5) as ptp,
            tc.tile_pool(name="an", bufs=2) as anp,
            tc.tile_pool(name="ps", bufs=1, space="PSUM") as psp,
            tc.tile_pool(name="dram", bufs=1, space="DRAM") as dpool,
        ):
            tm_sb = cpool.tile([P, P], f16)
            idm_sb = cpool.tile([P, P], f16)
            wq_sb = cpool.tile([P, KD, INNER_C], f16)
            wk_sb = cpool.tile([P, KD, INNER_C], f16)
            wv_sb = cpool.tile([P, KD, INNER_C], f16)
            wo_sb = cpool.tile([P, KD, DIM], f16)
            if apply_g:
                g_sb = cpool.tile([P, DIM], f32)
            xt = [cpool.tile([P, KD, SEQ], f16, name=f"xt{b}") for b in range(B)]

            # Q^T/K^T [128 = 2 heads x 64 dims, 2048 seq] per batch
            QT = [proj.tile([P, SEQ], f16, name=f"QT{b}") for b in range(B)]
            KT = [proj.tile([P, SEQ], f16, name=f"KT{b}") for b in range(B)]
            # V' [128 seq within kt, kt, head, 65] per batch; col 64 is the
            # ones column so AV col 64 accumulates the softmax denominator
            v_sb = [
                proj.tile([P, NKT, H_PER_CORE, DH + 1], f16, name=f"V{b}")
                for b in range(B)
            ]

            # per-core A2A block: 2 heads x 64 dims = 128 rows per unit
            ag_in = dpool.tile([N_CORES * P, 512], f16)
            ag_out = dpool.tile([N_CORES * P, 512], f16)

            # PSUM (8 banks): one [128,1024] ring x3 bufs = 6 banks shared
            # by scores / projections / transposes / stage-D out-proj;
            # av0/av1 [128,512] = 2 banks ([q, 4 x 65] AV accumulators,
            # col 65*qq+64 is the denominator)
            def ps_tile(name):
                return psp.tile([P, 1024], f32, tag="ps", bufs=3, name=name)

            def av_tile(hl, name):
                return psp.tile([P, 512], f32, tag=f"av{hl}", bufs=1,
                                name=name)

            # ---- stage A: x chunk DMA + projections for (batch, chunk) ----
            def stage_a(b, ch):
                sl = slice(ch * 512, (ch + 1) * 512)
                xr = xT[b].rearrange("(kd p) s -> p kd s", p=P)
                if b == 0 and ch == 0:
                    # wq + x gate the first matmul group: x in 4 quarter
                    # DMAs so early kd slices land while later ones stream;
                    # wk on the ACT queue so it doesn't queue behind x
                    nc.sync.dma_start(wq_sb[:], wq_d)
                    for k0 in range(0, KD, 2):
                        nc.sync.dma_start(
                            xt[b][:, k0 : k0 + 2, sl], xr[:, k0 : k0 + 2, sl]
                        )
                    nc.scalar.dma_start(wk_sb[:], wk_d)
                    nc.scalar.dma_start(wv_sb[:], wv_d)
                    nc.scalar.dma_start(tm_sb[:], tm_d)
                    nc.scalar.dma_start(idm_sb[:], idm_d)
                    for bb in range(B):
                        nc.vector.memset(v_sb[bb][:, :, :, DH : DH + 1], 1.0)
                else:
                    nc.sync.dma_start(xt[b][:, 0:4, sl], xr[:, 0:4, sl])
                    nc.sync.dma_start(xt[b][:, 4:8, sl], xr[:, 4:8, sl])
                if b == 1 and ch == 0:
                    # stage-D constants deferred to the batch-1 phase: the
                    # DMA queue idles under B(b0) and they must not sit
                    # ahead of x chunks 1-3 in the queue
                    nc.sync.dma_start(wo_sb[:], wo_d)
                    if apply_g:
                        nc.sync.dma_start(
                            g_sb[:], g_d[None, :].to_broadcast((P, DIM))
                        )

            # filler steps yield (approx_pe_ns, ) granules so stage B can
            # pull PE work sized to the per-kt exp-pacing deficit
            def emit_proj(b, ch):
                """q/k projections for column chunk ch of batch b; yields
                after every 2 contraction chunks (~427ns of PE work)."""
                sl = slice(ch * 512, (ch + 1) * 512)
                for wsb, dst in ((wq_sb, QT[b]), (wk_sb, KT[b])):
                    ps = ps_tile(f"pp{b}_{ch}")[:, :512]
                    for kd in range(KD):
                        nc.tensor.matmul(
                            ps,
                            wsb[:, kd, :],
                            xt[b][:, kd, sl],
                            start=(kd == 0),
                            stop=(kd == KD - 1),
                        )
                        if kd == KD - 1:
                            # copy emitted WITH the final step so it isn't
                            # deferred to a later (possibly boundary) pull
                            nc.vector.tensor_copy(dst[:, sl], ps)
                        if kd % 2 == 1:
                            yield 427

            def emit_v(b, ch):
                """V for the 4 seq blocks of chunk ch: V[sblk] [128, 128]."""
                for j in range(4):
                    s = 4 * ch + j
                    ps = ps_tile(f"pv{b}_{s}")[:, :INNER_C]
                    for kd in range(KD):
                        nc.tensor.matmul(
                            ps,
                            xt[b][:, kd, s * P : (s + 1) * P],
                            wv_sb[:, kd, :],
                            start=(kd == 0),
                            stop=(kd == KD - 1),
                        )
                    nc.vector.tensor_copy(
                        v_sb[b][:, s, :, 0:DH],
                        ps.rearrange("p (h d) -> p h d", h=H_PER_CORE),
                    )
                    yield 427

            # ---- stage B for one (batch, q-chunk), with PE fillers ----
            def stage_b(b, qc, pool, pending_tail=None):
                def fill(credit):
                    """Pull ~credit ns of filler PE work from the global
                    pool (the exp-pacing deficit of this kt)."""
                    while credit > 0 and pool:
                        try:
                            credit -= next(pool[0])
                        except StopIteration:
                            pool.pop(0)
                    return credit

                kmax = 4 * qc + 4
                av = [av_tile(hl, f"av{b}_{qc}_{hl}") for hl in range(2)]
                pts = {}

                def emit_av(kt):
                    qq_min = max(0, kt - 4 * qc)
                    pt = pts.pop(kt)
                    cols = 0
                    for hl in range(H_PER_CORE):
                        for qq in range(qq_min, 4):
                            # ONE start per bank: start_tensor_calc marks
                            # the whole 2KB zero region pending-zero, so a
                            # start per qq-region would wipe the sibling
                            # regions' accumulated data. The single start
                            # (kt0, qq0) marks the bank; each region's
                            # first write then lands as a replace.
                            nc.tensor.matmul(
                                av[hl][:, 65 * qq : 65 * qq + DH + 1],
                                pt[:, 512 * hl + 128 * qq :
                                   512 * hl + 128 * qq + 128],
                                v_sb[b][:, kt, hl, :],
                                start=(kt == 0 and qq == 0),
                                stop=(kt == 4 * qc + qq),
                                skip_group_check=True,
                            )
                            cols += DH + 1
                    return cols * 0.4167

                # software pipeline: scores(kt) is emitted ahead of AV(kt-1)
                # so the in-order PE queue runs scores(kt) while the ACT
                # engine is still computing exp(kt-1)
                for kt in range(kmax):
                    c0 = max(0, P * (kt - 4 * qc))
                    pe_ns = 0.0
                    sc = ps_tile(f"sc{b}_{qc}_{kt}")
                    for hl in range(H_PER_CORE):
                        hb = DH * hl
                        nc.tensor.matmul(
                            sc[:, 512 * hl + c0 : 512 * hl + 512],
                            KT[b][hb : hb + DH, kt * P : (kt + 1) * P],
                            QT[b][hb : hb + DH,
                                  qc * 512 + c0 : (qc + 1) * 512],
                            start=True,
                            stop=True,
                        )
                        pe_ns += (512 - c0) * 0.4167
                    if kt > 1:
                        # depth-2 software pipeline: at qc start, AV(0) waits
                        # for the previous chunk's av drain; keeping two
                        # score units queued ahead of it in the in-order PE
                        # FIFO hides that wait from the exp cadence
                        pe_ns += emit_av(kt - 2)
                    pt = ptp.tile([P, 1024], f16, tag="pt",
                                  name=f"pt{b}_{qc}_{kt}")
                    if c0 > 0:
                        # exp only the causally-valid columns of each head
                        scv = sc.rearrange("p (h q) -> p h q", h=2)
                        ptv = pt.rearrange("p (h q) -> p h q", h=2)
                        nc.scalar.activation(
                            ptv[:, :, c0:512], scv[:, :, c0:512], EXP
                        )
                    else:
                        nc.scalar.activation(pt[:], sc[:], EXP)
                    exp_ns = (2 * (512 - c0) + 250) * 0.8333
                    if kt - 4 * qc >= 0:
                        for hl in range(H_PER_CORE):
                            nc.vector.tensor_mul(
                                pt[:, 512 * hl + c0 : 512 * hl + c0 + P],
                                pt[:, 512 * hl + c0 : 512 * hl + c0 + P],
                                tm_sb[:],
                            )
                    pts[kt] = pt
                    fill(exp_ns - pe_ns)
                    if kt == 1 and pending_tail is not None:
                        # the previous unit's transposes/copies/DMA are
                        # emitted only now, behind this unit's first two
                        # exps, so the next exp doesn't queue behind a PE
                        # transpose that waits on the DVE normalize chain
                        pending_tail()
                emit_av(kmax - 2)
                emit_av(kmax - 1)
                # queue independent PE work ahead of the tail chain so the
                # PE isn't idle while the DVE runs reciprocal + normalize
                fill(1200)

                # ---- unit tail: normalize, transpose to [d, q], stage ----
                # The whole chain is deferred into the next unit's early
                # iterations so this unit's DVE ops don't block the next
                # unit's projection copies in the DVE queue.
                def tail():
                    # rc[q, qq] = 1 / denominator (column 65*qq+64 of av)
                    rc = [
                        anp.tile([P, 4], f32, tag=f"rc{hl}",
                                 name=f"rc{b}_{qc}_{hl}")
                        for hl in range(2)
                    ]
                    for hl in range(2):
                        nc.vector.reciprocal(
                            rc[hl][:], av[hl][:, DH : 4 * (DH + 1) : DH + 1]
                        )
                    # normalized attn in [q, d] f16 per (head, q-128-chunk)
                    aq = [
                        anp.tile([P, 4, DH], f16, tag=f"aq{hl}",
                                 name=f"aq{b}_{qc}_{hl}")
                        for hl in range(2)
                    ]
                    for hl in range(2):
                        for qq in range(4):
                            nc.vector.tensor_scalar_mul(
                                aq[hl][:, qq, :],
                                av[hl][:, 65 * qq : 65 * qq + DH],
                                rc[hl][:, qq : qq + 1],
                            )
                    # PE transposes back to [d, q] via matmul against the
                    # identity (aq^T = aq.T @ I; same 128-cycle cost, f32
                    # psum output): all 8 into one slot, partitions 0..63
                    tr = ps_tile(f"tr{b}_{qc}")
                    for hl in range(2):
                        for qq in range(4):
                            nc.tensor.matmul(
                                tr[0:DH, 512 * hl + 128 * qq :
                                   512 * hl + 128 * qq + 128],
                                aq[hl][:, qq, :],
                                idm_sb[:],
                                start=True,
                                stop=True,
                                skip_group_check=True,
                            )
                    # copy (partition shift for head 1) into the A2A block;
                    # DVE, because ACT (exp) is the binding stage-B engine
                    an = anp.tile([P, 512], f16, tag="an", name=f"an{b}_{qc}")
                    for hl in range(2):
                        nc.vector.tensor_copy(
                            an[DH * hl : DH * hl + DH, :],
                            tr[0:DH, 512 * hl : 512 * hl + 512],
                        )
                    row = P * (4 * b + qc)
                    nc.sync.dma_start(ag_in[row : row + P, :], an[:])

                return tail

            # ---- emission: interleave A chunks ahead of B iterations ----
            # Unit order ends with (1, 0): the last unit has no filler work
            # left, so its exp-pacing deficit is exposed — make it the
            # smallest unit (4 key tiles) instead of a 16-tile one. Chunk
            # projection order stays sequential; (1, 1) runs only after
            # chunk (1, 1) drained, which the keep-2 rule guarantees.
            units = [(b, qc) for b in range(B) for qc in range(NQC)]
            appends = [[units[u + 1]] if u + 1 < len(units) else []
                       for u in range(len(units))]
            stage_a(0, 0)
            for gen in (emit_proj(0, 0), emit_v(0, 0)):
                for _ in gen:
                    pass
            pending = None
            pool = []
            for u, (b, qc) in enumerate(units):
                keep = 0
                for cb, cc in appends[u]:
                    stage_a(cb, cc)
                    pool.append(emit_proj(cb, cc))
                    pool.append(emit_v(cb, cc))
                    keep += 2
                # every chunk this unit reads must be fully emitted before
                # its scores; only the just-appended generators may remain
                while len(pool) > keep:
                    for _ in pool.pop(0):
                        pass
                pending = stage_b(b, qc, pool, pending)
            pending()  # last unit's tail

            # ---- stage C: global 8-way AllToAll ----
            if use_collective:
                nc.gpsimd.collective_compute(
                    "AllToAll",
                    mybir.AluOpType.bypass,
                    replica_groups=[list(range(N_CORES))],
                    ins=[ag_in.opt()],
                    outs=[ag_out.opt()],
                )
            else:
                # 8 parallel block copies approximate the A2A's DMA
                # parallelism better than one serial 1MB copy
                for u in range(N_CORES):
                    nc.sync.dma_start(
                        ag_out[u * P : (u + 1) * P, :],
                        ag_in[u * P : (u + 1) * P, :],
                    )

            # ---- stage D: out-proj + layernorm on my 512 query rows ----
            with tc.tile_pool(name="staged", bufs=1) as sdp:
                at_sb = sdp.tile([P, KD, 512], f16)
                # block ic of ag_out (rows 128*ic..+128) holds heads
                # {2ic, 2ic+1}: exactly at_sb[:, ic, :]
                # preload the sqrt activation table (stage-B ran exp; the
                # switch costs ~1.3us and must not sit on the first LN chain)
                tbl = sdp.tile([1, 1], f32, name="tbl")
                nc.vector.memset(tbl[:], 1.0)
                nc.scalar.sqrt(tbl[:], tbl[:])
                agr = ag_out.rearrange("(ic p) q -> p ic q", p=P)
                nc.sync.dma_start(at_sb[:, 0:4, :], agr[:, 0:4, :])
                # second half from the gpsimd queue so the loads transfer
                # in parallel
                nc.gpsimd.dma_start(at_sb[:, 4:8, :], agr[:, 4:8, :])
                def chain(mt, pso, st):
                    """Combine half-stats, rsqrt, apply, store for tile mt.
                    Deferred one tile so its latency hides under the next
                    tile's matmuls (the tile scheduler replays emission
                    order, so the order here IS the execution order)."""
                    nm, vs = st[0], st[1]
                    # nm <- -(sum_a+sum_b)/D; ms <- (vs_a+vs_b)/D;
                    # sd <- sqrt(ms - nm^2 + EPS)  (DVE except the sqrt)
                    nmc = st[2][:, 0:1]
                    nc.vector.tensor_scalar(
                        nmc, nm[:, 0:1], nm[:, 1:2], -1.0 / DIM,
                        mybir.AluOpType.add, mybir.AluOpType.mult,
                    )
                    ms = st[3][:, 0:1]
                    nc.vector.tensor_scalar(
                        ms, vs[:, 0:1], vs[:, 1:2], 1.0 / DIM,
                        mybir.AluOpType.add, mybir.AluOpType.mult,
                    )
                    nm2 = st[3][:, 1:2]
                    nc.vector.tensor_mul(nm2, nmc, nmc)
                    sd = st[2][:, 1:2]
                    nc.vector.tensor_scalar(
                        sd, ms, nm2, EPS,
                        mybir.AluOpType.subtract, mybir.AluOpType.add,
                    )
                    nc.scalar.sqrt(sd, sd)
                    rs = st[1][:, 0:1]
                    nc.vector.reciprocal(rs, sd)
                    # (pso + nm) * rs == pso*rs + nm*rs: ACT passes with
                    # per-partition scale/bias APs, split by halves so the
                    # first output DMA overlaps the second apply
                    nmrs = st[2][:, 1:2]
                    nc.vector.tensor_mul(nmrs, nmc, rs)
                    o_sb = sdp.tile([P, DIM], f32, tag="osb", bufs=2,
                                    name=f"osb{mt}")
                    for nch in range(2):
                        half = slice(nch * 512, (nch + 1) * 512)
                        nc.scalar.activation(
                            o_sb[:, half], pso[:, half], IDENT,
                            bias=nmrs, scale=rs,
                        )
                        if apply_g:
                            nc.vector.tensor_mul(
                                o_sb[:, half], o_sb[:, half], g_sb[:, half]
                            )
                        eng = nc.sync if nch == 0 else nc.gpsimd
                        eng.dma_start(
                            out_d[mt * P : (mt + 1) * P, half],
                            o_sb[:, half],
                        )

                chains = []
                for mt in range(4):
                    pso = ps_tile(f"pd{mt}")
                    st = [
                        sdp.tile([P, 2], f32, tag="stat", bufs=12,
                                 name=f"st{mt}_{i}")
                        for i in range(4)
                    ]
                    sq = sdp.tile([P, DIM], f32, tag="sq", bufs=2,
                                  name=f"sq{mt}")
                    # per-half stats overlap the second matmul group: sum on
                    # DVE concurrently with sum-of-squares on ACT (the sq
                    # writes themselves are dead stores)
                    nm, vs = st[0], st[1]
                    for nch in range(2):
                        half = slice(nch * 512, (nch + 1) * 512)
                        for ic in range(KD):
                            nc.tensor.matmul(
                                pso[:, half],
                                at_sb[:, ic, mt * P : (mt + 1) * P],
                                wo_sb[:, ic, half],
                                start=(ic == 0),
                                stop=(ic == KD - 1),
                                skip_group_check=True,
                            )
                        nc.vector.reduce_sum(
                            nm[:, nch : nch + 1], pso[:, half], axis=AX
                        )
                        # sum of squares on ACT (the HW forbids two PSUM
                        # operands on a DVE tensor-tensor op); the sq
                        # writes themselves are dead stores
                        nc.scalar.activation(
                            sq[:, half], pso[:, half],
                            mybir.ActivationFunctionType.Square,
                            accum_out=vs[:, nch : nch + 1],
                        )
                        if nch == 0 and chains:
                            chains.pop(0)()
                    chains.append(
                        (lambda m, p, s: lambda: chain(m, p, s))(mt, pso, st)
                    )
                for c in chains:
                    c()

    nc.compile()
    return nc


_NC_CACHE = {}


def _get_nc(apply_g):
    key = ("nc", apply_g)
    if key not in _NC_CACHE:
        _NC_CACHE[key] = build_nc(apply_g=apply_g)
    return _NC_CACHE[key]


def make_in_maps(x, w_qkv, w_out, g):
    x = np.asarray(x, dtype=np.float32)
    w_qkv = np.asarray(w_qkv, dtype=np.float32)
    w_out = np.asarray(w_out, dtype=np.float32)
    g = np.asarray(g, dtype=np.float32)
    apply_g = not np.all(g == 1.0)

    xT0 = np.ascontiguousarray(x[0].T).astype(np.float16)
    xT1 = np.ascontiguousarray(x[1].T).astype(np.float16)

    def _prearrange(w):
        # [(ko p), m] -> [p, ko, m] so the SBUF load is one contiguous DMA
        return np.ascontiguousarray(
            w.reshape(KD, P, w.shape[1]).transpose(1, 0, 2)
        )

    wo = _prearrange(w_out.astype(np.float16))
    tm = np.triu(np.ones((P, P), dtype=np.float16))
    idm = np.eye(P, dtype=np.float16)

    in_maps = []
    for c in range(N_CORES):
        lo = c * INNER_C  # first inner column of this core's 2 heads
        wq = _prearrange(
            (w_qkv[:, lo : lo + INNER_C] * SCALE).astype(np.float16)
        )
        wk = _prearrange(
            w_qkv[:, DIM + lo : DIM + lo + INNER_C].astype(np.float16)
        )
        wv = _prearrange(
            w_qkv[:, 2 * DIM + lo : 2 * DIM + lo + INNER_C].astype(np.float16)
        )
        m = {
            "xT0": xT0,
            "xT1": xT1,
            "wq": wq,
            "wk": wk,
            "wv": wv,
            "wo": wo,
            "tm": tm,
            "idm": idm,
        }
        if apply_g:
            m["g"] = g
        in_maps.append(m)
    return in_maps


def assemble(results):
    out = np.empty((B, SEQ, DIM), dtype=np.float32)
    for c in range(N_CORES):
        b, r = divmod(c, 4)
        out[b, 512 * r : 512 * (r + 1), :] = results[c]["out"]
    return out


def _make_fast_runner(nc):
    """Cached PJRT runner for repeat kernel() calls: same execute path that
    run_bass_kernel_spmd uses under axon, but the jitted executable and the
    replicated device-resident inputs persist across calls."""
    import jax
    from jax.sharding import Mesh, PartitionSpec
    from jax.experimental.shard_map import shard_map
    from concourse.bass2jax import (
        _bass_exec_p, install_neuronx_cc_hook, partition_id_tensor,
    )

    install_neuronx_cc_hook()
    partition_name = nc.partition_id_tensor.name if nc.partition_id_tensor else None
    in_names, out_names, out_avals, zero_shapes = [], [], [], []
    for alloc in nc.m.functions[0].allocations:
        if not isinstance(alloc, mybir.MemoryLocationSet):
            continue
        name = alloc.memorylocations[0].name
        if alloc.kind == "ExternalInput":
            if name != partition_name:
                in_names.append(name)
        elif alloc.kind == "ExternalOutput":
            out_names.append(name)
            shape = tuple(alloc.tensor_shape)
            dtype = mybir.dt.np(alloc.dtype)
            out_avals.append(jax.core.ShapedArray(shape, dtype))
            zero_shapes.append((shape, dtype))
    n_params = len(in_names)
    n_outs = len(out_avals)
    all_names = in_names + out_names + ([partition_name] if partition_name else [])
    donate = tuple(range(n_params, n_params + n_outs))

    def _body(*args):
        operands = list(args)
        if partition_name is not None:
            operands.append(partition_id_tensor())
        return tuple(
            _bass_exec_p.bind(
                *operands,
                out_avals=tuple(out_avals),
                in_names=tuple(all_names),
                out_names=tuple(out_names),
                lowering_input_output_aliases=(),
                sim_require_finite=True,
                sim_require_nnan=True,
                nc=nc,
            )
        )

    devices = jax.devices()[:N_CORES]
    mesh = Mesh(np.asarray(devices), ("core",))
    sharded = jax.jit(
        shard_map(
            _body,
            mesh=mesh,
            in_specs=(PartitionSpec("core"),) * (n_params + n_outs),
            out_specs=(PartitionSpec("core"),) * n_outs,
            check_rep=False,
        ),
        donate_argnums=donate,
        keep_unused=True,
    )

    def run(in_maps):
        concat_in = [
            np.concatenate(
                [np.asarray(in_maps[c][nm]) for c in range(N_CORES)], axis=0
            )
            for nm in in_names
        ]
        zeros = [
            np.zeros((N_CORES * sh[0], *sh[1:]), dt) for sh, dt in zero_shapes
        ]
        outs = sharded(*concat_in, *zeros)
        full = np.asarray(outs[0]).reshape(N_CORES, *out_avals[0].shape)
        return [{out_names[0]: full[c]} for c in range(N_CORES)]

    return run


def kernel(x, mask, w_qkv, w_out, g):
    in_maps = make_in_maps(x, w_qkv, w_out, g)
    apply_g = "g" in in_maps[0]
    nc = _get_nc(apply_g)
    rkey = ("runner", apply_g)
    if rkey in _NC_CACHE:
        return assemble(_NC_CACHE[rkey](in_maps))
    res = bass_utils.run_bass_kernel_spmd(
        nc, in_maps, core_ids=list(range(N_CORES))
    )
    _NC_CACHE[rkey] = _make_fast_runner(nc)
    return assemble(res.results)
